# revision 17
# baseline (speedup 1.0000x reference)
"""MoE (top-2 of 8 experts, d=1024) — load-balanced hidden-split Bass kernel
for 8 trn2 cores.

Sharding (refinement of the sharding_hint's expert-parallel scheme): each
expert's MLP is split in half along the HIDDEN dimension (512 units each),
giving 16 shards. Shards are paired onto cores so that a big expert always
shares a core with a small expert: experts sorted by routed-token count,
pair p = (desc[p], desc[7-p]); core 2p takes hidden-half 0 of both, core
2p+1 takes hidden-half 1. Every core runs an identical program with token
capacities (CA, CB) = (padded max big count, padded max small count) — about
(2304+2048) slots instead of the 2*2304 a pure expert-parallel layout needs,
a ~9% PE-cycle cut. Partial y outputs (each half contributes a full-d
partial sum) are combined on the host, which also applies the per-token
top-2 gate weights (the y = yh0 + yh1 sum and the combine are both linear).

Device kernel (per core): for shard S in {A, B}:
    yS_partial = relu(xS @ W1S + b1S) @ W2S (+ b2 on half-0 cores only)
entirely in bf16 (PE runs bf16 at 1 cycle/row like float32r, but HBM
traffic halves; max rel err vs the fp32 reference ~4e-3, inside the 2e-2
gate). fp32 PSUM accumulation.

Device-side details:
 - all DRAM tensors packed 128-partition-major so every DMA moves 8-16KB
   contiguous rows (tiny descriptor counts)
 - software-pipelined chunks across both shards: chunk s's layer-1 matmuls
   interleave with chunk s-1's layer-2 matmuls at 128-column granularity
 - tail chunks processed first within each shard so the iteration ends on a
   full 512-token layer-2 phase, hiding the next iteration's weight reloads
 - layer-1 bias+relu on DVE, layer-2 bias on Act — splits elementwise load
"""

import numpy as np
import ml_dtypes

import concourse.bass as bass
import concourse.mybir as mybir
import concourse.tile as tile
from concourse import bacc
from concourse.bass_utils import run_bass_kernel_spmd

# Problem shapes (hardcoded per contract)
D = 1024   # d_model == d_hidden
HD = 512   # hidden half per shard
N_EXPERTS = 8
TOP_K = 2
N_CORES = 8
B, T = 4, 2048
N_TOKENS = B * T

F32 = mybir.dt.float32
BF16 = mybir.dt.bfloat16
BF = ml_dtypes.bfloat16
KC = D // 128    # layer-1 contraction chunks (8)
KH = HD // 128   # layer-2 contraction chunks (4) == layer-1 output chunks
MC = D // 128    # layer-2 output chunks (8)
NT = 512         # tokens per matmul (moving free dim; one PSUM bank fp32)
CGRAIN = 64      # capacity granularity


def chunk_list(C):
    """Chunk sizes in processing order: tail (if any) first, then 512s."""
    assert C % CGRAIN == 0 and C >= NT
    sizes = [NT] * (C // NT)
    if C % NT:
        sizes = [C % NT] + sizes
    return sizes


def build_moe_expert_kernel(C, repeat: int = 1, split_w: int = 2,
                            hoist_in: bool = False,
                            unroll: int = 1) -> bacc.Bacc:
    """Two half-expert shards A (cap CA) and B (cap CB), C = (CA, CB).

    DRAM inputs (packed partition-major, bf16 except biases):
      xA [nA, 128, KC, NT] (+ xAt [128, KC, tailA]), same for B
      wA1 [128, KC, HD], wA2 [128, KH, D], bA1 [128, KH], bA2 [128, MC]
      (same for B)
    Outputs: yA [nA, 128, MC, NT] (+ yAt), yB likewise (partial sums).
    `repeat` wraps the body in a hardware loop (slope-based HW timing).
    """
    CA, CB = C[0], C[1]
    nc = bacc.Bacc("TRN2", target_bir_lowering=False, debug=False,
                   num_devices=N_CORES)

    shards = []
    for sname, cap in (("A", CA), ("B", CB)):
        sizes = chunk_list(cap)
        nfull = sum(1 for s in sizes if s == NT)
        tail = cap % NT
        sd = {
            "name": sname, "sizes": sizes, "tail": tail, "nfull": nfull,
            "x": nc.dram_tensor(f"x{sname}", [nfull, 128, KC, NT], BF16,
                                kind="ExternalInput"),
            "w1": nc.dram_tensor(f"w{sname}1", [128, KC, HD], BF16,
                                 kind="ExternalInput"),
            "b1": nc.dram_tensor(f"b{sname}1", [128, KH], F32,
                                 kind="ExternalInput"),
            "w2": nc.dram_tensor(f"w{sname}2", [128, KH, D], BF16,
                                 kind="ExternalInput"),
            "b2": nc.dram_tensor(f"b{sname}2", [128, MC], F32,
                                 kind="ExternalInput"),
            "y": nc.dram_tensor(f"y{sname}", [nfull, 128, MC, NT], BF16,
                                kind="ExternalOutput"),
        }
        if tail:
            sd["xt"] = nc.dram_tensor(f"x{sname}t", [128, KC, tail], BF16,
                                      kind="ExternalInput")
            sd["yt"] = nc.dram_tensor(f"y{sname}t", [128, MC, tail], BF16,
                                      kind="ExternalOutput")
        shards.append(sd)

    def x_view(sd, n):
        # chunk n in processing order; tail (if any) is chunk 0
        if sd["tail"]:
            return sd["xt"].ap() if n == 0 else sd["x"].ap()[n - 1]
        return sd["x"].ap()[n]

    def y_view(sd, n):
        if sd["tail"]:
            return sd["yt"].ap() if n == 0 else sd["y"].ap()[n - 1]
        return sd["y"].ap()[n]

    # pipeline stages: (shard, chunk) in processing order
    stages = [(sd, n) for sd in shards for n in range(len(sd["sizes"]))]
    nst = len(stages)

    with tile.TileContext(nc) as tc:
        with (
            tc.tile_pool(name="weights", bufs=1) as wpool,
            tc.tile_pool(name="consts", bufs=1) as cpool,
            tc.tile_pool(name="xin",
                         bufs=(nst if hoist_in else 3)) as xpool,
            tc.tile_pool(name="hmid", bufs=2) as hpool,
            tc.tile_pool(name="yout", bufs=2) as ypool,
            tc.tile_pool(name="ph", bufs=3, space="PSUM") as phpool,
            tc.tile_pool(name="py", bufs=3, space="PSUM") as pypool,
        ):
            from contextlib import nullcontext
            loop_cm = (
                tc.For_i(0, repeat, 1,
                         hint_engines=(mybir.EngineType.PE,
                                       mybir.EngineType.Activation,
                                       mybir.EngineType.DVE,
                                       mybir.EngineType.SP))
                if repeat > 1 else nullcontext()
            )
            state: dict = {}

            def alloc_tiles():
                for sd in shards:
                    s = sd["name"]
                    state[f"w1{s}"] = wpool.tile([128, KC, HD], BF16,
                                                 tag=f"w1{s}",
                                                 name=f"w1{s}_sb")
                    state[f"w2{s}"] = wpool.tile([128, KH, D], BF16,
                                                 tag=f"w2{s}",
                                                 name=f"w2{s}_sb")
                    state[f"b1{s}"] = cpool.tile([128, KH], F32,
                                                 tag=f"b1{s}",
                                                 name=f"b1{s}_sb")
                    state[f"b2{s}"] = cpool.tile([128, MC], F32,
                                                 tag=f"b2{s}",
                                                 name=f"b2{s}_sb")

            def emit_h_mc(sd, n, mc, x_sb, h_sb):
                sz = sd["sizes"][n]
                s = sd["name"]
                ph = phpool.tile([128, NT], F32, tag="ph", name="ph")
                for kc in range(KC):
                    nc.tensor.matmul(
                        ph[:, :sz],
                        state[f"w1{s}"][:, kc, bass.ts(mc, 128)],
                        x_sb[:, kc, :sz],
                        start=(kc == 0), stop=(kc == KC - 1),
                    )
                # h = relu(ph + b1)   (DVE, PSUM -> SBUF bf16)
                nc.vector.tensor_scalar(
                    h_sb[:, mc, :sz], ph[:, :sz],
                    state[f"b1{s}"][:, mc:mc + 1], 0.0,
                    mybir.AluOpType.add, mybir.AluOpType.max,
                )

            def emit_y_mc(sd, n, mc, h_sb, y_sb):
                sz = sd["sizes"][n]
                s = sd["name"]
                py = pypool.tile([128, NT], F32, tag="py", name="py")
                for kh in range(KH):
                    nc.tensor.matmul(
                        py[:, :sz],
                        state[f"w2{s}"][:, kh, bass.ts(mc, 128)],
                        h_sb[:, kh, :sz],
                        start=(kh == 0), stop=(kh == KH - 1),
                    )
                # y = py + b2   (Act engine, PSUM -> SBUF bf16)
                nc.scalar.activation(
                    y_sb[:, mc, :sz], py[:, :sz],
                    mybir.ActivationFunctionType.Identity,
                    bias=state[f"b2{s}"][:, mc:mc + 1],
                )

            def emit_w_dma(sd, which):
                s = sd["name"]
                if which == 1:
                    nc.sync.dma_start(state[f"w1{s}"][:], sd["w1"].ap())
                    nc.sync.dma_start(state[f"b1{s}"][:], sd["b1"].ap())
                else:
                    nc.sync.dma_start(state[f"w2{s}"][:], sd["w2"].ap())
                    nc.sync.dma_start(state[f"b2{s}"][:], sd["b2"].ap())

            def emit_prologue(x_tiles):
                # Interleaved wA1/x0 DMAs so the first matmuls wait only on
                # their own slices.
                sd0, n0 = stages[0]
                g = KC // split_w
                for i in range(split_w):
                    ks = slice(i * g, (i + 1) * g)
                    nc.sync.dma_start(state["w1A"][:, ks, :],
                                      sd0["w1"].ap()[:, ks, :])
                    nc.sync.dma_start(x_tiles[0][:, ks, :sd0["sizes"][n0]],
                                      x_view(sd0, n0)[:, ks, :])
                nc.sync.dma_start(state["b1A"][:], sd0["b1"].ap())

            def emit_pipeline(x_tiles, prefetch):
                h_tiles = {}
                y_tiles = {}
                for si in range(nst + 1):
                    cur = stages[si] if si < nst else None
                    prev = stages[si - 1] if si > 0 else None
                    if prefetch and si + 1 < nst:  # prefetch next stage's x
                        sdn, nn = stages[si + 1]
                        xt = xpool.tile([128, KC, NT], BF16, tag="x",
                                        name=f"xs{si + 1}")
                        nc.sync.dma_start(xt[:, :, :sdn["sizes"][nn]],
                                          x_view(sdn, nn))
                        x_tiles[si + 1] = xt
                    if cur is not None:
                        h_tiles[si] = hpool.tile([128, KH, NT], BF16,
                                                 tag="h", name=f"hs{si}")
                    if prev is not None:
                        y_tiles[si - 1] = ypool.tile([128, MC, NT], BF16,
                                                     tag="y", name=f"ys{si}")
                    for mc in range(MC):
                        if cur is not None and mc < KH:
                            emit_h_mc(cur[0], cur[1], mc, x_tiles[si],
                                      h_tiles[si])
                        if prev is not None:
                            emit_y_mc(prev[0], prev[1], mc, h_tiles[si - 1],
                                      y_tiles[si - 1])
                    if prev is not None:
                        # single writeback per chunk: 128 contiguous 8KB rows
                        sdp, np_ = prev
                        nc.sync.dma_start(
                            y_view(sdp, np_)[:, :, :],
                            y_tiles[si - 1][:, :, :sdp["sizes"][np_]])
                    if not hoist_in:
                        # spread remaining weight DMAs across early stages
                        if si == 0:
                            emit_w_dma(shards[0], 2)
                        elif si == 1:
                            emit_w_dma(shards[1], 1)
                        elif si == 2:
                            emit_w_dma(shards[1], 2)
                    if prefetch:
                        x_tiles.pop(si - 1, None)
                    h_tiles.pop(si - 2, None)
                    y_tiles.pop(si - 2, None)

            if hoist_in:
                alloc_tiles()
                for sd in shards:
                    emit_w_dma(sd, 1)
                    emit_w_dma(sd, 2)
                x_tiles = {}
                for si, (sd, n) in enumerate(stages):
                    x_tiles[si] = xpool.tile([128, KC, NT], BF16, tag="x",
                                             name=f"xh{si}")
                    nc.sync.dma_start(x_tiles[si][:, :, :sd["sizes"][n]],
                                      x_view(sd, n))
                with loop_cm:
                    emit_pipeline(x_tiles, prefetch=False)
            else:
                with loop_cm:
                    for _ in range(unroll):
                        alloc_tiles()
                        x0 = xpool.tile([128, KC, NT], BF16, tag="x",
                                        name="x0")
                        x_tiles = {0: x0}
                        emit_prologue(x_tiles)
                        emit_pipeline(x_tiles, prefetch=True)

    nc.compile()
    return nc


_NC_CACHE: dict = {}


def _get_kernel(C, repeat: int = 1, **opts) -> bacc.Bacc:
    key = (C, repeat, tuple(sorted(opts.items())))
    if key not in _NC_CACHE:
        _NC_CACHE[key] = build_moe_expert_kernel(C, repeat, **opts)
    return _NC_CACHE[key]


def _pad(n):
    return max(NT, ((n + CGRAIN - 1) // CGRAIN) * CGRAIN)


def dispatch(x, W_gate, b_gate):
    """Host-side gate + top-2 dispatch plan. Returns (xf, ids, wts, C)."""
    xf = np.ascontiguousarray(np.asarray(x).reshape(-1, D), dtype=np.float32)
    scores = xf @ np.asarray(W_gate, np.float32) + np.asarray(b_gate, np.float32)
    # top-2 expert ids per token (order irrelevant: contributions are summed)
    top2 = np.argpartition(scores, N_EXPERTS - TOP_K, axis=1)[:, -TOP_K:]
    ids, wts = [], []
    for e in range(N_EXPERTS):
        tok = np.nonzero((top2 == e).any(axis=1))[0]
        ids.append(tok)
        wts.append(scores[tok, e])
    counts = [len(t) for t in ids]
    order = list(np.argsort(-np.asarray(counts), kind="stable"))
    # pair p: (big, small) -> cores 2p (hidden half 0) and 2p+1 (half 1)
    pairs = [(int(order[p]), int(order[7 - p])) for p in range(4)]
    CA = _pad(max(counts[a] for a, _ in pairs))
    CB = _pad(max(counts[b] for _, b in pairs))
    return xf, ids, wts, (CA, CB, tuple(pairs))


def pack_rows(a):
    """[(kc kp), n] row-major -> [128, nkc, n] partition-major."""
    nkc = a.shape[0] // 128
    return np.ascontiguousarray(a.reshape(nkc, 128, -1).transpose(1, 0, 2))


def _pack_x(xTe, cap):
    """xT [D, cnt] -> packed chunk blocks (tail chunk first)."""
    Dd, cnt = xTe.shape
    xp = np.zeros((128, KC, cap), BF)
    xp[:, :, :cnt] = pack_rows(xTe)
    tail = cap % NT
    nfull = cap // NT
    xb = np.ascontiguousarray(
        xp[:, :, tail:].reshape(128, KC, nfull, NT).transpose(2, 0, 1, 3))
    xt = np.ascontiguousarray(xp[:, :, :tail]) if tail else None
    return xb, xt


def make_in_maps(parts, xf, ids, wts, C):
    """Build per-core input dicts (packed partition-major bf16 blocks)."""
    W1, b1, W2, b2 = parts
    CA, CB, pairs = C
    in_maps = []
    for p in range(4):
        for h in range(2):
            hs = slice(h * HD, (h + 1) * HD)
            m = {}
            for sname, cap, e in (("A", CA, pairs[p][0]),
                                  ("B", CB, pairs[p][1])):
                xTe = xf[ids[e]].T.astype(BF)
                xb, xt = _pack_x(xTe, cap)
                m[f"x{sname}"] = xb
                if xt is not None:
                    m[f"x{sname}t"] = xt
                m[f"w{sname}1"] = pack_rows(
                    np.asarray(W1[e][:, hs], np.float32).astype(BF))
                m[f"w{sname}2"] = pack_rows(
                    np.asarray(W2[e][hs, :], np.float32).astype(BF))
                m[f"b{sname}1"] = np.ascontiguousarray(
                    np.asarray(b1[e][hs], np.float32).reshape(KH, 128).T)
                b2v = (np.asarray(b2[e], np.float32) if h == 0
                       else np.zeros(D, np.float32))
                m[f"b{sname}2"] = np.ascontiguousarray(
                    b2v.reshape(MC, 128).T)
            in_maps.append(m)
    return in_maps


def _unpack_y(r, sname, cap):
    """packed y blocks -> yT [D, cap] fp32 (tail chunk first)."""
    tail = cap % NT
    nfull = cap // NT
    yb = r[f"y{sname}"].transpose(2, 1, 0, 3).reshape(D, nfull * NT)
    if tail:
        yt = r[f"y{sname}t"].transpose(1, 0, 2).reshape(D, tail)
        yb = np.concatenate([yt, yb], axis=1)
    return yb.astype(np.float32)


def kernel(x, W_gate, b_gate, W1, b1, W2, b2):
    xf, ids, wts, C = dispatch(x, W_gate, b_gate)
    CA, CB, pairs = C
    nc = _get_kernel(C)

    in_maps = make_in_maps((W1, b1, W2, b2), xf, ids, wts, C)
    res = run_bass_kernel_spmd(nc, in_maps, core_ids=list(range(N_CORES)))

    out = np.zeros((N_TOKENS, D), np.float32)
    for p in range(4):
        r0, r1 = res.results[2 * p], res.results[2 * p + 1]
        for sname, cap, e in (("A", CA, pairs[p][0]),
                              ("B", CB, pairs[p][1])):
            cnt = len(ids[e])
            yT = _unpack_y(r0, sname, cap) + _unpack_y(r1, sname, cap)
            out[ids[e]] += yT.T[:cnt] * wts[e][:, None]
    return out.reshape(B, T, D)
